# revision 4
# baseline (speedup 1.0000x reference)
"""Trainium2 Bass kernel for nn_DistractorScorer (sparse_attention).

Strategy
--------
Data-parallel over batch B=16 across 8 NeuronCores (2 batches/core); the
distractor dim N=32 and all params are replicated per core.

Per core the device program computes, entirely on-chip:
  scores1 = MLP([ft | fd] @ W1 + b1) @ W2 + b2                (PE + ACT + DVE)
  inners  = Ttgt @ Ddst^T (contraction over D=2048, fp32 PE)  per i-group
  masked row/col maxes -> two softmaxes (tw over X, dw over Y)
     - row path: free-axis segmented reduce + DVE 32x32 block transposes
     - col path: PE transposes (via identity) + free-axis reduces
  target_feats side folded as  tw @ (Ttgt @ oW1a)  (Q-matrix trick)
  distr_feats side folded as   segmented sum_y dw*D  (DVE) -> @ oW1b
  scores += MLP2, then per-batch log_softmax on device.

Host-side work is limited to input marshalling: fp32 mask->additive-bias
conversion and laying tensors out exactly as SBUF wants them
(feature-major, partition-outer) so every big DMA is contiguous.

Wbil is checked against identity (it is identity in setup_inputs); a
non-identity Wbil or a fully-masked row falls back to an exact numpy
implementation of the reference.
"""

import os
import sys
import types

import numpy as np

try:  # pragma: no cover - environment shim
    import concourse.bass as bass
except ImportError:  # pragma: no cover
    sys.path.insert(0, "/opt/trn_rl_repo")
    import concourse.bass as bass

import concourse.tile as tile
from concourse import mybir
from concourse.bass_utils import run_bass_kernel_spmd
from concourse.masks import make_identity
from contextlib import ExitStack

# ---------------- problem constants (hardcoded per contract) ----------------
B, N, X, D, H = 16, 32, 36, 2048, 512
NCORES = 8
BPC = B // NCORES           # batches per core = 2
ROWS = BPC * N              # 64 (b, i) rows per core
CH = D // 128               # 16 feature chunks
CH2 = 2 * CH                # 32 chunks over 2D
GI = 8                      # distractors per group
NG = N // GI                # 4 groups
GW = GI * X                 # 288 columns per group
NEG = -1.0e9
f32 = mybir.dt.float32

# ---------------------------------------------------------------------------
# Workarounds for this container's walrus build: only ONE inline sync-wait is
# accepted per instruction. Split Tile's assigned waits into standalone
# EventSemaphore instructions (same engine, just before the instruction), and
# split the kernel-tail Drain the same way.
# ---------------------------------------------------------------------------
_MAX_WAITS = 1
_uid = [0]
_patched = [False]


def _split_waits_in_place(instructions_by_block):
    for bb_name, insts in instructions_by_block.items():
        new_list = []
        for inst in insts:
            si = getattr(inst, "sync_info", None)
            waits = list(si.on_wait) if (si and si.on_wait) else []
            if len(waits) > _MAX_WAITS:
                keep = waits[:_MAX_WAITS]
                for w in waits[_MAX_WAITS:]:
                    _uid[0] += 1
                    new_list.append(
                        mybir.InstEventSemaphore(
                            name=f"{inst.name}-wsplit{_uid[0]}",
                            engine=inst.engine,
                            ins=[],
                            outs=[],
                            sync_info=mybir.SyncInfo(on_wait=[w], on_update=[]),
                        )
                    )
                si.on_wait = keep
            new_list.append(inst)
        instructions_by_block[bb_name] = new_list


def _apply_patches():
    if _patched[0]:
        return
    _patched[0] = True

    orig_postorder = tile.postorder_instruction_blocks

    def postorder_with_split(instructions, start_bb, output):
        _split_waits_in_place(instructions)
        return orig_postorder(instructions, start_bb, output)

    tile.postorder_instruction_blocks = postorder_with_split

    def drain_and_barrier_split(self, tick_clock, wait_clock):
        from concourse.vector_clock import ScopedClock

        drain_inst = self.nc.sync.drain()
        wait_clock.add_sem_waits(
            drain_inst.ins, ScopedClock({None: tick_clock.global_clock})
        )
        si = drain_inst.ins.sync_info
        waits = list(si.on_wait or [])
        if len(waits) > _MAX_WAITS:
            si.on_wait = waits[:_MAX_WAITS]
            rest = waits[_MAX_WAITS:]
            while rest:
                chunk, rest = rest[:_MAX_WAITS], rest[_MAX_WAITS:]
                extra = self.nc.sync.drain()
                esi = extra.ins.sync_info
                if esi is None:
                    extra.ins.sync_info = mybir.SyncInfo(on_wait=chunk, on_update=[])
                else:
                    esi.on_wait = chunk

        self.nc.all_engine_barrier()
        assert self.sems is not None
        popped = self.nc._tile_sem_poison_stack.pop()
        assert popped is self._sem_poison
        self.nc.clear_and_free_semaphores(list(self.sems.allocated().values()))
        self.nc.all_engine_barrier()

    tile.TileContext._drain_and_barrier = drain_and_barrier_split


def _bcast_free(ap, insert_counts):
    """Insert step-0 free dims. ap free dims stay; insert_counts is a list of
    (position, count) into the FREE dims (0 = right after partitions)."""
    new_ap = [list(ap.ap[0])]
    free = [list(d) for d in ap.ap[1:]]
    for pos, count in sorted(insert_counts, reverse=True):
        free.insert(pos, [0, count])
    return bass.AP(tensor=ap.tensor, offset=ap.offset, ap=new_ap + free)


# ---------------------------------------------------------------------------
# Device program (built once per process)
# ---------------------------------------------------------------------------
_PROGRAM = None


def _build_program():
    _apply_patches()
    nc = bass.Bass()
    AF = mybir.ActivationFunctionType
    OP = mybir.AluOpType
    AX = mybir.AxisListType

    d_cat = nc.declare_dram_parameter("cat_t", [128, CH2, ROWS], f32, isOutput=False)
    d_w1 = nc.declare_dram_parameter("w1_t", [128, CH2, H], f32, isOutput=False)
    d_ow1 = nc.declare_dram_parameter("ow1_t", [128, CH2, H], f32, isOutput=False)
    d_tt = nc.declare_dram_parameter("tt_t", [BPC, 128, CH, X], f32, isOutput=False)
    d_dt = nc.declare_dram_parameter("dt_t", [BPC, NG, 128, CH, GW], f32, isOutput=False)
    d_b1 = nc.declare_dram_parameter("b1_r", [1, H], f32, isOutput=False)
    d_ob1 = nc.declare_dram_parameter("ob1_r", [1, H], f32, isOutput=False)
    d_w2 = nc.declare_dram_parameter("w2_r", [1, H], f32, isOutput=False)
    d_ow2 = nc.declare_dram_parameter("ow2_r", [1, H], f32, isOutput=False)
    d_bias2 = nc.declare_dram_parameter("bias2_r", [1, 1], f32, isOutput=False)
    d_mtb = nc.declare_dram_parameter("mtb", [BPC, X, 1], f32, isOutput=False)
    d_mdb = nc.declare_dram_parameter("mdb", [BPC, 1, N * X], f32, isOutput=False)
    d_out = nc.declare_dram_parameter("out", [1, ROWS], f32, isOutput=True)

    with tile.TileContext(nc) as tc, ExitStack() as ctx:
        const = ctx.enter_context(tc.tile_pool(name="const", bufs=1))
        work = ctx.enter_context(tc.tile_pool(name="work", bufs=2))

        ident = const.tile([128, 128], f32, tag="ident")
        make_identity(nc, ident)
        ones128 = const.tile([1, 128], f32, tag="ones128")
        nc.vector.memset(ones128, 1.0)

        w2bc = const.tile([ROWS, H], f32, tag="w2bc")
        nc.sync.dma_start(out=w2bc, in_=d_w2[:].to_broadcast((ROWS, H)))
        ow2bc = const.tile([ROWS, H], f32, tag="ow2bc")
        nc.sync.dma_start(out=ow2bc, in_=d_ow2[:].to_broadcast((ROWS, H)))
        bias2bc = const.tile([ROWS, 1], f32, tag="bias2bc")
        nc.sync.dma_start(out=bias2bc, in_=d_bias2[:].to_broadcast((ROWS, 1)))

        mtcol = const.tile([X, BPC], f32, tag="mtcol")
        nc.sync.dma_start(out=mtcol, in_=d_mtb[:].rearrange("b x o -> x (b o)"))
        mdsb = []
        for b in range(BPC):
            t = const.tile([X, N * X], f32, tag=f"mdsb{b}", name=f"mdsb{b}")
            nc.sync.dma_start(out=t, in_=d_mdb[b].to_broadcast((X, N * X)))
            mdsb.append(t)
        ttsb = []
        for b in range(BPC):
            t = const.tile([128, CH, X], f32, tag=f"ttsb{b}", name=f"ttsb{b}")
            nc.sync.dma_start(out=t, in_=d_tt[b])
            ttsb.append(t)

        s1 = const.tile([ROWS, 1], f32, tag="s1")
        dft = const.tile([128, CH, ROWS], f32, tag="dft")
        qsb = [const.tile([X, H], f32, tag=f"qsb{b}", name=f"qsb{b}") for b in range(BPC)]
        twb = [const.tile([64, N], f32, tag=f"twb{b}", name=f"twb{b}") for b in range(BPC)]

        # ---------------- phase 0: fc-scorer MLP ----------------
        with tc.tile_pool(name="w1p", bufs=1) as w1p:
            w1sb = w1p.tile([128, CH2, H], f32, tag="w1sb")
            nc.sync.dma_start(out=w1sb, in_=d_w1[:])
            catsb = w1p.tile([128, CH2, ROWS], f32, tag="catsb")
            nc.sync.dma_start(out=catsb, in_=d_cat[:])
            b1sb = w1p.tile([1, H], f32, tag="b1sb")
            nc.sync.dma_start(out=b1sb, in_=d_b1[:])
            with tc.tile_pool(name="ps0", bufs=1, space="PSUM") as ps0:
                h1ps = ps0.tile([ROWS, H], f32, tag="h1ps")
                for c in range(CH2):
                    nc.tensor.matmul(
                        h1ps[:], catsb[:, c, :], w1sb[:, c, :],
                        start=(c == 0), stop=False,
                    )
                nc.tensor.matmul(
                    h1ps[:], ones128[:, 0:ROWS], b1sb[:], start=False, stop=True
                )
                hr1 = work.tile([ROWS, H], f32, tag="hrelu")
                nc.scalar.activation(out=hr1[:], in_=h1ps[:], func=AF.Relu)
            prod1 = work.tile([ROWS, H], f32, tag="prod")
            nc.vector.tensor_tensor(
                out=prod1[:], in0=hr1[:], in1=w2bc[:], op=OP.mult
            )
            nc.vector.tensor_reduce(s1[:], prod1[:], AX.X, OP.add)

        # ---------------- oW1 + Q matrices ----------------
        ow1sb = const.tile([128, CH2, H], f32, tag="ow1sb")
        nc.sync.dma_start(out=ow1sb, in_=d_ow1[:])
        ob1sb = const.tile([1, H], f32, tag="ob1sb")
        nc.sync.dma_start(out=ob1sb, in_=d_ob1[:])

        with tc.tile_pool(name="psq", bufs=2, space="PSUM") as psq:
            for b in range(BPC):
                qps = psq.tile([X, H], f32, tag="qps")
                for c in range(CH):
                    nc.tensor.matmul(
                        qps[:], ttsb[b][:, c, :], ow1sb[:, c, :],
                        start=(c == 0), stop=(c == CH - 1),
                    )
                nc.vector.tensor_copy(out=qsb[b][:], in_=qps[:])

        # ---------------- main loop ----------------
        dpool = ctx.enter_context(tc.tile_pool(name="dpool", bufs=3))
        wdpool = ctx.enter_context(tc.tile_pool(name="wdpool", bufs=1))
        psI = ctx.enter_context(tc.tile_pool(name="psI", bufs=2, space="PSUM"))
        psT = ctx.enter_context(tc.tile_pool(name="psT", bufs=1, space="PSUM"))
        psR = ctx.enter_context(tc.tile_pool(name="psR", bufs=1, space="PSUM"))
        psW = ctx.enter_context(tc.tile_pool(name="psW", bufs=1, space="PSUM"))
        psH = ctx.enter_context(tc.tile_pool(name="psH", bufs=1, space="PSUM"))

        h2ps = psH.tile([ROWS, H], f32, tag="h2ps")

        CM_CHUNKS = [(0, 108), (108, 108), (216, 72)]

        for b in range(BPC):
            rmb = work.tile([64, N], f32, tag="rmb")
            nc.vector.memset(rmb, NEG)
            for g in range(NG):
                dtg = dpool.tile([128, CH, GW], f32, tag="dtg")
                nc.sync.dma_start(out=dtg, in_=d_dt[b, g])

                psumI = psI.tile([X, GW], f32, tag="psumI")
                for c in range(CH):
                    nc.tensor.matmul(
                        psumI[:], ttsb[b][:, c, :], dtg[:, c, :],
                        start=(c == 0), stop=(c == CH - 1),
                    )
                # masked scores M = inners + mt_bias (rows) + md_bias (cols)
                mg = work.tile([X, GW], f32, tag="mg")
                nc.vector.tensor_scalar(
                    out=mg[:], in0=psumI[:], scalar1=mtcol[:, b : b + 1],
                    scalar2=None, op0=OP.add,
                )
                nc.vector.tensor_tensor(
                    out=mg[:], in0=mg[:],
                    in1=mdsb[b][:, g * GW : (g + 1) * GW], op=OP.add,
                )
                # row path: max over y per (x, i)
                nc.vector.tensor_reduce(
                    rmb[0:X, g * GI : (g + 1) * GI],
                    mg[:].rearrange("p (i y) -> p i y", y=X),
                    AX.X, OP.max,
                )
                # col path: max over x per (i, y) via PE transposes
                cmrow = work.tile([1, GW], f32, tag="cmrow")
                for off, w in CM_CHUNKS:
                    ptT = psT.tile([108, X], f32, tag="ptT")
                    nc.tensor.transpose(
                        ptT[0:w, :], mg[:, off : off + w], ident[0:X, 0:X]
                    )
                    ccol = work.tile([108, 1], f32, tag="ccol")
                    nc.vector.tensor_reduce(
                        ccol[0:w, :], ptT[0:w, :], AX.X, OP.max
                    )
                    ptr = psR.tile([1, 108], f32, tag="ptr")
                    nc.tensor.transpose(
                        ptr[:, 0:w], ccol[0:w, :], ident[0:w, 0:w]
                    )
                    nc.vector.tensor_copy(
                        out=cmrow[:, off : off + w], in_=ptr[:, 0:w]
                    )
                # dw softmax over y per i (single-partition row ops)
                cmax = work.tile([1, GI], f32, tag="cmax")
                cm3 = cmrow[:].rearrange("p (i y) -> p i y", y=X)
                nc.vector.tensor_reduce(cmax[:], cm3, AX.X, OP.max)
                erow = work.tile([1, GW], f32, tag="erow")
                nc.vector.tensor_tensor(
                    out=erow[:].rearrange("p (i y) -> p i y", y=X),
                    in0=cm3, in1=cmax[:].to_broadcast((1, GI, X)),
                    op=OP.subtract,
                )
                nc.scalar.activation(out=erow[:], in_=erow[:], func=AF.Exp)
                ssum = work.tile([1, GI], f32, tag="ssum")
                nc.vector.tensor_reduce(
                    ssum[:], erow[:].rearrange("p (i y) -> p i y", y=X),
                    AX.X, OP.add,
                )
                srec = work.tile([1, GI], f32, tag="srec")
                nc.vector.reciprocal(out=srec[:], in_=ssum[:])
                dwrow = work.tile([1, GW], f32, tag="dwrow")
                nc.vector.tensor_tensor(
                    out=dwrow[:].rearrange("p (i y) -> p i y", y=X),
                    in0=erow[:].rearrange("p (i y) -> p i y", y=X),
                    in1=srec[:].to_broadcast((1, GI, X)), op=OP.mult,
                )
                # broadcast dw across 128 partitions via ones-matmul
                dwps = psW.tile([128, GW], f32, tag="dwps")
                nc.tensor.matmul(dwps[:], ones128[:], dwrow[:], start=True, stop=True)
                # weighted sum over y: d_feats^T chunk
                wd = wdpool.tile([128, CH, GW], f32, tag="wd")
                dw4 = _bcast_free(
                    dwps[:].rearrange("p (i y) -> p i y", y=X), [(0, CH)]
                )
                nc.vector.tensor_tensor(
                    out=wd[:].rearrange("p c (i y) -> p c i y", y=X),
                    in0=dtg[:].rearrange("p c (i y) -> p c i y", y=X),
                    in1=dw4, op=OP.mult,
                )
                nc.vector.tensor_reduce(
                    dft[:, :, b * N + g * GI : b * N + (g + 1) * GI],
                    wd[:].rearrange("p c (i y) -> p c i y", y=X),
                    AX.X, OP.add,
                )
            # ---- tw softmax over x (DVE 32x32 block transposes) ----
            rmT = work.tile([N, 64], f32, tag="rmT")
            nc.vector.transpose(rmT[:, 0:32], rmb[0:32, :])
            nc.vector.transpose(rmT[:, 32:64], rmb[32:64, :])
            tmx = work.tile([N, 1], f32, tag="tmx")
            nc.vector.tensor_reduce(tmx[:], rmT[:], AX.X, OP.max)
            te = work.tile([N, 64], f32, tag="te")
            nc.vector.tensor_scalar(
                out=te[:], in0=rmT[:], scalar1=tmx[:], scalar2=None,
                op0=OP.subtract,
            )
            nc.scalar.activation(out=te[:], in_=te[:], func=AF.Exp)
            tsum = work.tile([N, 1], f32, tag="tsum")
            nc.vector.tensor_reduce(tsum[:], te[:], AX.X, OP.add)
            trec = work.tile([N, 1], f32, tag="trec")
            nc.vector.reciprocal(out=trec[:], in_=tsum[:])
            twT = work.tile([N, 64], f32, tag="twT")
            nc.vector.tensor_scalar(
                out=twT[:], in0=te[:], scalar1=trec[:], scalar2=None, op0=OP.mult
            )
            nc.vector.transpose(twb[b][0:32, :], twT[:, 0:32])
            nc.vector.transpose(twb[b][32:64, :], twT[:, 32:64])
            # target-side contribution: tw @ Q[b]
            nc.tensor.matmul(
                h2ps[b * N : (b + 1) * N, :], twb[b][0:X, :], qsb[b][:],
                start=True, stop=False,
            )

        # ---------------- MLP2 tail ----------------
        for c in range(CH):
            nc.tensor.matmul(
                h2ps[:], dft[:, c, :], ow1sb[:, CH + c, :],
                start=False, stop=False,
            )
        nc.tensor.matmul(
            h2ps[:], ones128[:, 0:ROWS], ob1sb[:], start=False, stop=True
        )
        hr2 = work.tile([ROWS, H], f32, tag="hrelu")
        nc.scalar.activation(out=hr2[:], in_=h2ps[:], func=AF.Relu)
        prod2 = work.tile([ROWS, H], f32, tag="prod")
        s2 = work.tile([ROWS, 1], f32, tag="s2")
        nc.vector.tensor_tensor(
            out=prod2[:], in0=hr2[:], in1=ow2bc[:], op=OP.mult
        )
        nc.vector.tensor_reduce(s2[:], prod2[:], AX.X, OP.add)
        s = work.tile([ROWS, 1], f32, tag="s")
        nc.vector.tensor_tensor(out=s[:], in0=s1[:], in1=s2[:], op=OP.add)
        nc.vector.tensor_tensor(out=s[:], in0=s[:], in1=bias2bc[:], op=OP.add)

        # ---------------- per-batch log_softmax over i ----------------
        with tc.tile_pool(name="psF", bufs=1, space="PSUM") as psF:
            ptf = psF.tile([1, ROWS], f32, tag="ptf")
            nc.tensor.transpose(ptf[:], s[:], ident[0:ROWS, 0:ROWS])
            srow = work.tile([1, ROWS], f32, tag="srow")
            nc.vector.tensor_copy(out=srow[:], in_=ptf[:])
        s3 = srow[:].rearrange("p (b i) -> p b i", i=N)
        fmx = work.tile([1, BPC], f32, tag="fmx")
        nc.vector.tensor_reduce(fmx[:], s3, AX.X, OP.max)
        t1 = work.tile([1, ROWS], f32, tag="t1")
        nc.vector.tensor_tensor(
            out=t1[:].rearrange("p (b i) -> p b i", i=N),
            in0=s3, in1=fmx[:].to_broadcast((1, BPC, N)), op=OP.subtract,
        )
        ef = work.tile([1, ROWS], f32, tag="ef")
        nc.scalar.activation(out=ef[:], in_=t1[:], func=AF.Exp)
        fsum = work.tile([1, BPC], f32, tag="fsum")
        nc.vector.tensor_reduce(
            fsum[:], ef[:].rearrange("p (b i) -> p b i", i=N), AX.X, OP.add
        )
        fln = work.tile([1, BPC], f32, tag="fln")
        nc.scalar.activation(out=fln[:], in_=fsum[:], func=AF.Ln)
        outsb = work.tile([1, ROWS], f32, tag="outsb")
        nc.vector.tensor_tensor(
            out=outsb[:].rearrange("p (b i) -> p b i", i=N),
            in0=t1[:].rearrange("p (b i) -> p b i", i=N),
            in1=fln[:].to_broadcast((1, BPC, N)), op=OP.subtract,
        )
        nc.sync.dma_start(out=d_out[:], in_=outsb[:])

    return nc


def _get_program():
    global _PROGRAM
    if _PROGRAM is None:
        _PROGRAM = _build_program()
    return _PROGRAM


# ---------------------------------------------------------------------------
# Host-side reference fallback (exact numpy port of the jax reference)
# ---------------------------------------------------------------------------
def _host_reference(fc_feats_target, fc_feats_distr, att_feats_target,
                    att_feats_distr, att_masks_target, att_masks_distr,
                    W1, b1, W2, b2, Wbil, oW1, ob1, oW2, ob2):
    def mlp(x, w1, bb1, w2, bb2):
        h = np.maximum(x @ w1 + bb1, 0.0)
        return (h @ w2 + bb2)[..., 0]

    ft = np.broadcast_to(fc_feats_target, fc_feats_distr.shape)
    cat = np.concatenate([ft, fc_feats_distr], axis=-1)
    scores = mlp(cat, W1, b1, W2, b2)
    tproj = np.einsum("bxd,de->bxe", att_feats_target[:, 0], Wbil)
    inners = np.einsum("bxd,biyd->bixy", tproj, att_feats_distr)
    mo = (att_masks_target[:, 0][:, None, :, None]
          * att_masks_distr[:, :, None, :])
    inners = np.where(mo > 0, inners, NEG)

    def sm(x):
        x = x - x.max(-1, keepdims=True)
        e = np.exp(x)
        return e / e.sum(-1, keepdims=True)

    tw = sm(inners.max(3))
    dw = sm(inners.max(2))
    tf = np.einsum("bxd,bix->bid", att_feats_target[:, 0], tw)
    df = np.einsum("biyd,biy->bid", att_feats_distr, dw)
    cat2 = np.concatenate([tf, df], axis=-1)
    scores = scores + mlp(cat2, oW1, ob1, oW2, ob2)
    ls = scores - scores.max(-1, keepdims=True)
    return (ls - np.log(np.exp(ls).sum(-1, keepdims=True))).astype(np.float32)


# ---------------------------------------------------------------------------
# NTFF trace support for test harnesses (KERNEL_TRACE=1)
# ---------------------------------------------------------------------------
def _install_trace_hook():
    import antenv

    if "antenv.axon_hooks" not in sys.modules:
        mod = types.ModuleType("antenv.axon_hooks")
        mod._hook = None

        def set_axon_ntff_profile_hook(h):
            mod._hook = h

        def get_axon_ntff_profile_hook():
            return mod._hook

        mod.set_axon_ntff_profile_hook = set_axon_ntff_profile_hook
        mod.get_axon_ntff_profile_hook = get_axon_ntff_profile_hook
        sys.modules["antenv.axon_hooks"] = mod
        antenv.axon_hooks = mod
    if sys.modules["antenv.axon_hooks"]._hook is None:
        from trn_agent_boot.trn_boot import _ntff_profile_via_ctypes

        hook = _ntff_profile_via_ctypes("/opt/axon/libaxon_pjrt.so")
        sys.modules["antenv.axon_hooks"].set_axon_ntff_profile_hook(hook)


# ---------------------------------------------------------------------------
# Host marshalling + entry point
# ---------------------------------------------------------------------------
def _prepare_in_maps(fc_feats_target, fc_feats_distr, att_feats_target,
                     att_feats_distr, att_masks_target, att_masks_distr,
                     W1, b1, W2, b2, oW1, ob1, oW2, ob2):
    fp = np.float32
    cat = np.concatenate(
        [np.broadcast_to(fc_feats_target, fc_feats_distr.shape), fc_feats_distr],
        axis=-1,
    ).astype(fp, copy=False)                       # [B, N, 2D]

    w1_t = np.ascontiguousarray(
        W1.astype(fp, copy=False).reshape(CH2, 128, H).transpose(1, 0, 2))
    ow1_t = np.ascontiguousarray(
        oW1.astype(fp, copy=False).reshape(CH2, 128, H).transpose(1, 0, 2))

    T = att_feats_target[:, 0].astype(fp, copy=False)       # [B, X, D]
    # [B, X, D] -> [B, 128, CH, X]
    tt_all = np.ascontiguousarray(
        T.transpose(0, 2, 1).reshape(B, CH, 128, X).transpose(0, 2, 1, 3))

    Dd = att_feats_distr.astype(fp, copy=False)             # [B, N, X, D]
    # -> [B, NG, 128, CH, GW]
    dt_all = np.ascontiguousarray(
        Dd.reshape(B, NG, GW, D).transpose(0, 1, 3, 2)
        .reshape(B, NG, CH, 128, GW).transpose(0, 1, 3, 2, 4))

    mtb = np.where(att_masks_target[:, 0] > 0, 0.0, NEG).astype(fp)  # [B, X]
    mdb = np.where(att_masks_distr > 0, 0.0, NEG).astype(fp)         # [B, N, X]

    b1_r = np.ascontiguousarray(b1.astype(fp).reshape(1, H))
    ob1_r = np.ascontiguousarray(ob1.astype(fp).reshape(1, H))
    w2_r = np.ascontiguousarray(W2.astype(fp).reshape(1, H))
    ow2_r = np.ascontiguousarray(oW2.astype(fp).reshape(1, H))
    bias2 = np.ascontiguousarray(
        (b2.astype(np.float64) + ob2.astype(np.float64)).astype(fp).reshape(1, 1))

    in_maps = []
    for cc in range(NCORES):
        sl = slice(cc * BPC, (cc + 1) * BPC)
        cat_c = cat[sl].reshape(ROWS, 2 * D)
        cat_t = np.ascontiguousarray(
            cat_c.T.reshape(CH2, 128, ROWS).transpose(1, 0, 2))
        in_maps.append({
            "cat_t": cat_t,
            "w1_t": w1_t,
            "ow1_t": ow1_t,
            "tt_t": np.ascontiguousarray(tt_all[sl]),
            "dt_t": np.ascontiguousarray(dt_all[sl]),
            "b1_r": b1_r,
            "ob1_r": ob1_r,
            "w2_r": w2_r,
            "ow2_r": ow2_r,
            "bias2_r": bias2,
            "mtb": np.ascontiguousarray(mtb[sl].reshape(BPC, X, 1)),
            "mdb": np.ascontiguousarray(mdb[sl].reshape(BPC, 1, N * X)),
        })
    return in_maps


def kernel(**inputs):
    inp = {k: np.asarray(v) for k, v in inputs.items()}

    ident_ok = np.array_equal(
        inp["Wbil"], np.eye(D, dtype=inp["Wbil"].dtype))
    masks_ok = bool(
        (inp["att_masks_target"][:, 0] != 0).any(axis=1).all()
        and (inp["att_masks_distr"] != 0).any(axis=2).all())
    if not (ident_ok and masks_ok):
        return _host_reference(**inp)

    in_maps = _prepare_in_maps(
        inp["fc_feats_target"], inp["fc_feats_distr"],
        inp["att_feats_target"], inp["att_feats_distr"],
        inp["att_masks_target"], inp["att_masks_distr"],
        inp["W1"], inp["b1"], inp["W2"], inp["b2"],
        inp["oW1"], inp["ob1"], inp["oW2"], inp["ob2"])

    nc = _get_program()
    trace = os.environ.get("KERNEL_TRACE", "") == "1"
    if trace:
        _install_trace_hook()
        res = run_bass_kernel_spmd(
            nc, in_maps, list(range(NCORES)), trace=True,
            trace_cores=list(range(NCORES)))
        print(f"HW exec time: {res.exec_time_ns} ns")
    else:
        res = run_bass_kernel_spmd(nc, in_maps, list(range(NCORES)))

    out = np.concatenate(
        [res.results[cc]["out"].reshape(BPC, N) for cc in range(NCORES)], axis=0)
    return out.astype(np.float32, copy=False)


# revision 5
# speedup vs baseline: 1.1821x; 1.1821x over previous
"""Trainium2 Bass kernel for nn_DistractorScorer (sparse_attention).

Strategy
--------
Data-parallel over batch B=16 across 8 NeuronCores (2 batches/core); the
distractor dim N=32 and all params are replicated per core.

Per core the device program computes, entirely on-chip:
  scores1 = MLP([ft | fd] @ W1 + b1) @ W2 + b2        (bf16 PE + ACT + DVE)
  inners  = Ttgt @ Ddst^T (contraction over D=2048, fp32 PE)  per i-group
  masked row/col maxes -> two softmaxes (tw over X, dw over Y)
     - row path: free-axis segmented reduce + DVE 32x32 block transposes
     - col path: PE transposes (via identity) + free-axis reduces
  target_feats side folded as  tw @ (Ttgt @ oW1a)  (Q-matrix trick, bf16)
  distr_feats side folded as   segmented sum_y dw*D  (DVE) -> @ oW1b
  scores += MLP2, then per-batch log_softmax on device.

The i-group pipeline is software-pipelined by one group: group g's
DVE/transpose postprocessing is emitted after group g+1's PE matmuls so
the in-order PE queue never stalls on the DVE chain.

The bilinear/attention path (inners) stays fp32 — bf16 there costs ~2.5e-2
final relative error; bf16 on the MLP paths costs ~1e-3 (measured).

Host-side work is limited to input marshalling: fp32 mask->additive-bias
conversion, bf16 casts, and laying tensors out exactly as SBUF wants them
(feature-major, partition-outer) so every big DMA is contiguous.

Wbil is checked against identity (it is identity in setup_inputs); a
non-identity Wbil or a fully-masked mask row falls back to an exact numpy
implementation of the reference.
"""

import os
import sys
import types

import numpy as np
import ml_dtypes

try:  # pragma: no cover - environment shim
    import concourse.bass as bass
except ImportError:  # pragma: no cover
    sys.path.insert(0, "/opt/trn_rl_repo")
    import concourse.bass as bass

import concourse.tile as tile
from concourse import mybir
from concourse.bass_utils import run_bass_kernel_spmd
from concourse.masks import make_identity
from contextlib import ExitStack

# ---------------- problem constants (hardcoded per contract) ----------------
B, N, X, D, H = 16, 32, 36, 2048, 512
NCORES = 8
BPC = B // NCORES           # batches per core = 2
ROWS = BPC * N              # 64 (b, i) rows per core
CH = D // 128               # 16 feature chunks
CH2 = 2 * CH                # 32 chunks over 2D
GI = 8                      # distractors per group
NG = N // GI                # 4 groups
GW = GI * X                 # 288 columns per group
NEG = -1.0e9
f32 = mybir.dt.float32
bf16 = mybir.dt.bfloat16
BF = ml_dtypes.bfloat16

# ---------------------------------------------------------------------------
# Workarounds for this container's walrus build: only ONE inline sync-wait is
# accepted per instruction. Split Tile's assigned waits into standalone
# EventSemaphore instructions (same engine, just before the instruction), and
# split the kernel-tail Drain the same way.
# ---------------------------------------------------------------------------
_MAX_WAITS = 1
_uid = [0]
_patched = [False]


def _split_waits_in_place(instructions_by_block):
    for bb_name, insts in instructions_by_block.items():
        new_list = []
        for inst in insts:
            si = getattr(inst, "sync_info", None)
            waits = list(si.on_wait) if (si and si.on_wait) else []
            if len(waits) > _MAX_WAITS:
                keep = waits[:_MAX_WAITS]
                for w in waits[_MAX_WAITS:]:
                    _uid[0] += 1
                    new_list.append(
                        mybir.InstEventSemaphore(
                            name=f"{inst.name}-wsplit{_uid[0]}",
                            engine=inst.engine,
                            ins=[],
                            outs=[],
                            sync_info=mybir.SyncInfo(on_wait=[w], on_update=[]),
                        )
                    )
                si.on_wait = keep
            new_list.append(inst)
        instructions_by_block[bb_name] = new_list


def _apply_patches():
    if _patched[0]:
        return
    _patched[0] = True

    orig_postorder = tile.postorder_instruction_blocks

    def postorder_with_split(instructions, start_bb, output):
        _split_waits_in_place(instructions)
        return orig_postorder(instructions, start_bb, output)

    tile.postorder_instruction_blocks = postorder_with_split

    def drain_and_barrier_split(self, tick_clock, wait_clock):
        from concourse.vector_clock import ScopedClock

        drain_inst = self.nc.sync.drain()
        wait_clock.add_sem_waits(
            drain_inst.ins, ScopedClock({None: tick_clock.global_clock})
        )
        si = drain_inst.ins.sync_info
        waits = list(si.on_wait or [])
        if len(waits) > _MAX_WAITS:
            si.on_wait = waits[:_MAX_WAITS]
            rest = waits[_MAX_WAITS:]
            while rest:
                chunk, rest = rest[:_MAX_WAITS], rest[_MAX_WAITS:]
                extra = self.nc.sync.drain()
                esi = extra.ins.sync_info
                if esi is None:
                    extra.ins.sync_info = mybir.SyncInfo(on_wait=chunk, on_update=[])
                else:
                    esi.on_wait = chunk

        self.nc.all_engine_barrier()
        assert self.sems is not None
        popped = self.nc._tile_sem_poison_stack.pop()
        assert popped is self._sem_poison
        self.nc.clear_and_free_semaphores(list(self.sems.allocated().values()))
        self.nc.all_engine_barrier()

    tile.TileContext._drain_and_barrier = drain_and_barrier_split


def _bcast_free(ap, insert_counts):
    """Insert step-0 free dims into an AP (position 0 = right after the
    partition dim)."""
    new_ap = [list(ap.ap[0])]
    free = [list(d) for d in ap.ap[1:]]
    for pos, count in sorted(insert_counts, reverse=True):
        free.insert(pos, [0, count])
    return bass.AP(tensor=ap.tensor, offset=ap.offset, ap=new_ap + free)


# ---------------------------------------------------------------------------
# Device program (built once per process)
# ---------------------------------------------------------------------------
_PROGRAM = None


def _build_program():
    _apply_patches()
    nc = bass.Bass()
    AF = mybir.ActivationFunctionType
    OP = mybir.AluOpType
    AX = mybir.AxisListType

    d_cat = nc.declare_dram_parameter("cat_t", [128, CH2, ROWS], bf16, isOutput=False)
    d_w1 = nc.declare_dram_parameter("w1_t", [128, CH2, H], bf16, isOutput=False)
    d_ow1a = nc.declare_dram_parameter("ow1a_t", [128, CH, H], bf16, isOutput=False)
    d_ow1b = nc.declare_dram_parameter("ow1b_t", [128, CH, H], f32, isOutput=False)
    d_tt = nc.declare_dram_parameter("tt_t", [BPC, 128, CH, X], f32, isOutput=False)
    d_tt2 = nc.declare_dram_parameter("tt2_t", [BPC, 128, CH, X], bf16, isOutput=False)
    d_dt = nc.declare_dram_parameter("dt_t", [BPC, NG, 128, CH, GW], f32, isOutput=False)
    d_b1 = nc.declare_dram_parameter("b1_r", [1, H], bf16, isOutput=False)
    d_ob1 = nc.declare_dram_parameter("ob1_r", [1, H], f32, isOutput=False)
    d_w2 = nc.declare_dram_parameter("w2_r", [1, H], f32, isOutput=False)
    d_ow2 = nc.declare_dram_parameter("ow2_r", [1, H], f32, isOutput=False)
    d_bias2 = nc.declare_dram_parameter("bias2_r", [1, 1], f32, isOutput=False)
    d_mtb = nc.declare_dram_parameter("mtb", [BPC, X, 1], f32, isOutput=False)
    d_mdb = nc.declare_dram_parameter("mdb", [BPC, 1, N * X], f32, isOutput=False)
    d_out = nc.declare_dram_parameter("out", [1, ROWS], f32, isOutput=True)

    CM_CHUNKS = [(0, 108), (108, 108), (216, 72)]

    with tile.TileContext(nc) as tc, ExitStack() as ctx:
        const = ctx.enter_context(tc.tile_pool(name="const", bufs=1))
        work = ctx.enter_context(tc.tile_pool(name="work", bufs=2))

        ident = const.tile([128, 128], f32, tag="ident")
        make_identity(nc, ident)
        ones128 = const.tile([1, 128], f32, tag="ones128")
        nc.vector.memset(ones128, 1.0)
        onesbf = const.tile([1, 128], bf16, tag="onesbf")
        nc.vector.memset(onesbf, 1.0)

        w2bc = const.tile([ROWS, H], f32, tag="w2bc")
        nc.sync.dma_start(out=w2bc, in_=d_w2[:].to_broadcast((ROWS, H)))
        ow2bc = const.tile([ROWS, H], f32, tag="ow2bc")
        nc.sync.dma_start(out=ow2bc, in_=d_ow2[:].to_broadcast((ROWS, H)))
        bias2bc = const.tile([ROWS, 1], f32, tag="bias2bc")
        nc.sync.dma_start(out=bias2bc, in_=d_bias2[:].to_broadcast((ROWS, 1)))

        mtcol = const.tile([X, BPC], f32, tag="mtcol")
        nc.sync.dma_start(out=mtcol, in_=d_mtb[:].rearrange("b x o -> x (b o)"))
        mdsb = []
        for b in range(BPC):
            t = const.tile([X, N * X], f32, tag=f"mdsb{b}", name=f"mdsb{b}")
            nc.sync.dma_start(out=t, in_=d_mdb[b].to_broadcast((X, N * X)))
            mdsb.append(t)
        ttsb = []
        tt2sb = []
        for b in range(BPC):
            t = const.tile([128, CH, X], f32, tag=f"ttsb{b}", name=f"ttsb{b}")
            nc.sync.dma_start(out=t, in_=d_tt[b])
            ttsb.append(t)
            t2 = const.tile([128, CH, X], bf16, tag=f"tt2sb{b}", name=f"tt2sb{b}")
            nc.sync.dma_start(out=t2, in_=d_tt2[b])
            tt2sb.append(t2)

        s1 = const.tile([ROWS, 1], f32, tag="s1")
        dft = const.tile([128, CH, ROWS], f32, tag="dft")
        qsb = [const.tile([X, H], f32, tag=f"qsb{b}", name=f"qsb{b}")
               for b in range(BPC)]
        twb = [const.tile([64, N], f32, tag=f"twb{b}", name=f"twb{b}")
               for b in range(BPC)]

        # ---------------- phase 0: fc-scorer MLP (bf16) ----------------
        with tc.tile_pool(name="w1p", bufs=1) as w1p:
            w1sb = w1p.tile([128, CH2, H], bf16, tag="w1sb")
            nc.sync.dma_start(out=w1sb, in_=d_w1[:])
            catsb = w1p.tile([128, CH2, ROWS], bf16, tag="catsb")
            nc.sync.dma_start(out=catsb, in_=d_cat[:])
            b1sb = w1p.tile([1, H], bf16, tag="b1sb")
            nc.sync.dma_start(out=b1sb, in_=d_b1[:])
            with tc.tile_pool(name="ps0", bufs=1, space="PSUM") as ps0:
                h1ps = ps0.tile([ROWS, H], f32, tag="h1ps")
                for c in range(CH2):
                    nc.tensor.matmul(
                        h1ps[:], catsb[:, c, :], w1sb[:, c, :],
                        start=(c == 0), stop=False,
                    )
                nc.tensor.matmul(
                    h1ps[:], onesbf[:, 0:ROWS], b1sb[:], start=False, stop=True
                )
                hr1 = work.tile([ROWS, H], f32, tag="hrelu")
                nc.scalar.activation(out=hr1[:], in_=h1ps[:], func=AF.Relu)
            prod1 = work.tile([ROWS, H], f32, tag="prod")
            nc.vector.tensor_tensor(
                out=prod1[:], in0=hr1[:], in1=w2bc[:], op=OP.mult
            )
            nc.vector.tensor_reduce(s1[:], prod1[:], AX.X, OP.add)

        # ---------------- oW1 + Q matrices (bf16) ----------------
        ow1asb = const.tile([128, CH, H], bf16, tag="ow1asb")
        nc.sync.dma_start(out=ow1asb, in_=d_ow1a[:])
        ow1bsb = const.tile([128, CH, H], f32, tag="ow1bsb")
        nc.sync.dma_start(out=ow1bsb, in_=d_ow1b[:])
        ob1sb = const.tile([1, H], f32, tag="ob1sb")
        nc.sync.dma_start(out=ob1sb, in_=d_ob1[:])

        with tc.tile_pool(name="psq", bufs=2, space="PSUM") as psq:
            for b in range(BPC):
                qps = psq.tile([X, H], f32, tag="qps")
                for c in range(CH):
                    nc.tensor.matmul(
                        qps[:], tt2sb[b][:, c, :], ow1asb[:, c, :],
                        start=(c == 0), stop=(c == CH - 1),
                    )
                nc.vector.tensor_copy(out=qsb[b][:], in_=qps[:])

        # ---------------- main loop (software-pipelined by 1 group) -------
        dpool = ctx.enter_context(tc.tile_pool(name="dpool", bufs=4))
        wdpool = ctx.enter_context(tc.tile_pool(name="wdpool", bufs=1))
        psI = ctx.enter_context(tc.tile_pool(name="psI", bufs=2, space="PSUM"))
        psT = ctx.enter_context(tc.tile_pool(name="psT", bufs=1, space="PSUM"))
        psR = ctx.enter_context(tc.tile_pool(name="psR", bufs=1, space="PSUM"))
        psW = ctx.enter_context(tc.tile_pool(name="psW", bufs=1, space="PSUM"))
        psH = ctx.enter_context(tc.tile_pool(name="psH", bufs=1, space="PSUM"))

        h2ps = psH.tile([ROWS, H], f32, tag="h2ps")
        rmb = {}

        def emit_inners(b, g):
            """PE-heavy front: stream D group and accumulate inners."""
            dtg = dpool.tile([128, CH, GW], f32, tag="dtg", name=f"dtg{b}_{g}")
            nc.sync.dma_start(out=dtg, in_=d_dt[b, g])
            psumI = psI.tile([X, GW], f32, tag="psumI", name=f"psumI{b}_{g}")
            for c in range(CH):
                nc.tensor.matmul(
                    psumI[:], ttsb[b][:, c, :], dtg[:, c, :],
                    start=(c == 0), stop=(c == CH - 1),
                )
            return dtg, psumI

        def emit_post(b, g, dtg, psumI):
            """DVE/ACT/PE-transpose tail for a finished inners group."""
            mg = work.tile([X, GW], f32, tag="mg", name=f"mg{b}_{g}")
            nc.vector.tensor_scalar(
                out=mg[:], in0=psumI[:], scalar1=mtcol[:, b : b + 1],
                scalar2=None, op0=OP.add,
            )
            nc.vector.tensor_tensor(
                out=mg[:], in0=mg[:],
                in1=mdsb[b][:, g * GW : (g + 1) * GW], op=OP.add,
            )
            # row path: max over y per (x, i)
            nc.vector.tensor_reduce(
                rmb[b][0:X, g * GI : (g + 1) * GI],
                mg[:].rearrange("p (i y) -> p i y", y=X),
                AX.X, OP.max,
            )
            # col path: max over x per (i, y) via PE transposes
            cmrow = work.tile([1, GW], f32, tag="cmrow", name=f"cmrow{b}_{g}")
            for off, w in CM_CHUNKS:
                ptT = psT.tile([108, X], f32, tag="ptT", name=f"ptT{b}_{g}_{off}")
                nc.tensor.transpose(
                    ptT[0:w, :], mg[:, off : off + w], ident[0:X, 0:X]
                )
                ccol = work.tile([108, 1], f32, tag="ccol", name=f"ccol{b}_{g}_{off}")
                nc.vector.tensor_reduce(ccol[0:w, :], ptT[0:w, :], AX.X, OP.max)
                ptr = psR.tile([1, 108], f32, tag="ptr", name=f"ptr{b}_{g}_{off}")
                nc.tensor.transpose(ptr[:, 0:w], ccol[0:w, :], ident[0:w, 0:w])
                nc.vector.tensor_copy(out=cmrow[:, off : off + w], in_=ptr[:, 0:w])
            # dw softmax over y per i (single-partition row ops)
            cmax = work.tile([1, GI], f32, tag="cmax", name=f"cmax{b}_{g}")
            cm3 = cmrow[:].rearrange("p (i y) -> p i y", y=X)
            nc.vector.tensor_reduce(cmax[:], cm3, AX.X, OP.max)
            erow = work.tile([1, GW], f32, tag="erow", name=f"erow{b}_{g}")
            nc.vector.tensor_tensor(
                out=erow[:].rearrange("p (i y) -> p i y", y=X),
                in0=cm3, in1=cmax[:].to_broadcast((1, GI, X)), op=OP.subtract,
            )
            nc.scalar.activation(out=erow[:], in_=erow[:], func=AF.Exp)
            ssum = work.tile([1, GI], f32, tag="ssum", name=f"ssum{b}_{g}")
            nc.vector.tensor_reduce(
                ssum[:], erow[:].rearrange("p (i y) -> p i y", y=X), AX.X, OP.add
            )
            srec = work.tile([1, GI], f32, tag="srec", name=f"srec{b}_{g}")
            nc.vector.reciprocal(out=srec[:], in_=ssum[:])
            dwrow = work.tile([1, GW], f32, tag="dwrow", name=f"dwrow{b}_{g}")
            nc.vector.tensor_tensor(
                out=dwrow[:].rearrange("p (i y) -> p i y", y=X),
                in0=erow[:].rearrange("p (i y) -> p i y", y=X),
                in1=srec[:].to_broadcast((1, GI, X)), op=OP.mult,
            )
            # broadcast dw across 128 partitions via ones-matmul
            dwps = psW.tile([128, GW], f32, tag="dwps", name=f"dwps{b}_{g}")
            nc.tensor.matmul(dwps[:], ones128[:], dwrow[:], start=True, stop=True)
            # weighted sum over y: d_feats^T columns for this group
            wd = wdpool.tile([128, CH, GW], f32, tag="wd", name=f"wd{b}_{g}")
            dw4 = _bcast_free(
                dwps[:].rearrange("p (i y) -> p i y", y=X), [(0, CH)]
            )
            nc.vector.tensor_tensor(
                out=wd[:].rearrange("p c (i y) -> p c i y", y=X),
                in0=dtg[:].rearrange("p c (i y) -> p c i y", y=X),
                in1=dw4, op=OP.mult,
            )
            nc.vector.tensor_reduce(
                dft[:, :, b * N + g * GI : b * N + (g + 1) * GI],
                wd[:].rearrange("p c (i y) -> p c i y", y=X),
                AX.X, OP.add,
            )

        def emit_tw(b):
            """Per-batch tw softmax over x + target-side MLP2 contribution."""
            rmT = work.tile([N, 64], f32, tag="rmT", name=f"rmT{b}")
            nc.vector.transpose(rmT[:, 0:32], rmb[b][0:32, :])
            nc.vector.transpose(rmT[:, 32:64], rmb[b][32:64, :])
            tmx = work.tile([N, 1], f32, tag="tmx", name=f"tmx{b}")
            nc.vector.tensor_reduce(tmx[:], rmT[:], AX.X, OP.max)
            te = work.tile([N, 64], f32, tag="te", name=f"te{b}")
            nc.vector.tensor_scalar(
                out=te[:], in0=rmT[:], scalar1=tmx[:], scalar2=None,
                op0=OP.subtract,
            )
            nc.scalar.activation(out=te[:], in_=te[:], func=AF.Exp)
            tsum = work.tile([N, 1], f32, tag="tsum", name=f"tsum{b}")
            nc.vector.tensor_reduce(tsum[:], te[:], AX.X, OP.add)
            trec = work.tile([N, 1], f32, tag="trec", name=f"trec{b}")
            nc.vector.reciprocal(out=trec[:], in_=tsum[:])
            twT = work.tile([N, 64], f32, tag="twT", name=f"twT{b}")
            nc.vector.tensor_scalar(
                out=twT[:], in0=te[:], scalar1=trec[:], scalar2=None, op0=OP.mult
            )
            nc.vector.transpose(twb[b][0:32, :], twT[:, 0:32])
            nc.vector.transpose(twb[b][32:64, :], twT[:, 32:64])
            nc.tensor.matmul(
                h2ps[b * N : (b + 1) * N, :], twb[b][0:X, :], qsb[b][:],
                start=True, stop=False,
            )

        sched = [(b, g) for b in range(BPC) for g in range(NG)]
        pending = None
        for b, g in sched:
            if g == 0:
                rmb[b] = work.tile([64, N], f32, tag="rmb", name=f"rmb{b}")
                nc.vector.memset(rmb[b], NEG)
            st = emit_inners(b, g)
            if pending is not None:
                pb, pg, pdtg, ppsum = pending
                emit_post(pb, pg, pdtg, ppsum)
                if pg == NG - 1:
                    emit_tw(pb)
            pending = (b, g, st[0], st[1])
        pb, pg, pdtg, ppsum = pending
        emit_post(pb, pg, pdtg, ppsum)
        emit_tw(pb)

        # ---------------- MLP2 tail ----------------
        for c in range(CH):
            nc.tensor.matmul(
                h2ps[:], dft[:, c, :], ow1bsb[:, c, :], start=False, stop=False
            )
        nc.tensor.matmul(
            h2ps[:], ones128[:, 0:ROWS], ob1sb[:], start=False, stop=True
        )
        hr2 = work.tile([ROWS, H], f32, tag="hrelu")
        nc.scalar.activation(out=hr2[:], in_=h2ps[:], func=AF.Relu)
        prod2 = work.tile([ROWS, H], f32, tag="prod")
        s2 = work.tile([ROWS, 1], f32, tag="s2")
        nc.vector.tensor_tensor(out=prod2[:], in0=hr2[:], in1=ow2bc[:], op=OP.mult)
        nc.vector.tensor_reduce(s2[:], prod2[:], AX.X, OP.add)
        s = work.tile([ROWS, 1], f32, tag="s")
        nc.vector.tensor_tensor(out=s[:], in0=s1[:], in1=s2[:], op=OP.add)
        nc.vector.tensor_tensor(out=s[:], in0=s[:], in1=bias2bc[:], op=OP.add)

        # ---------------- per-batch log_softmax over i ----------------
        with tc.tile_pool(name="psF", bufs=1, space="PSUM") as psF:
            ptf = psF.tile([1, ROWS], f32, tag="ptf")
            nc.tensor.transpose(ptf[:], s[:], ident[0:ROWS, 0:ROWS])
            srow = work.tile([1, ROWS], f32, tag="srow")
            nc.vector.tensor_copy(out=srow[:], in_=ptf[:])
        s3 = srow[:].rearrange("p (b i) -> p b i", i=N)
        fmx = work.tile([1, BPC], f32, tag="fmx")
        nc.vector.tensor_reduce(fmx[:], s3, AX.X, OP.max)
        t1 = work.tile([1, ROWS], f32, tag="t1")
        nc.vector.tensor_tensor(
            out=t1[:].rearrange("p (b i) -> p b i", i=N),
            in0=s3, in1=fmx[:].to_broadcast((1, BPC, N)), op=OP.subtract,
        )
        ef = work.tile([1, ROWS], f32, tag="ef")
        nc.scalar.activation(out=ef[:], in_=t1[:], func=AF.Exp)
        fsum = work.tile([1, BPC], f32, tag="fsum")
        nc.vector.tensor_reduce(
            fsum[:], ef[:].rearrange("p (b i) -> p b i", i=N), AX.X, OP.add
        )
        fln = work.tile([1, BPC], f32, tag="fln")
        nc.scalar.activation(out=fln[:], in_=fsum[:], func=AF.Ln)
        outsb = work.tile([1, ROWS], f32, tag="outsb")
        nc.vector.tensor_tensor(
            out=outsb[:].rearrange("p (b i) -> p b i", i=N),
            in0=t1[:].rearrange("p (b i) -> p b i", i=N),
            in1=fln[:].to_broadcast((1, BPC, N)), op=OP.subtract,
        )
        nc.sync.dma_start(out=d_out[:], in_=outsb[:])

    return nc


def _get_program():
    global _PROGRAM
    if _PROGRAM is None:
        _PROGRAM = _build_program()
    return _PROGRAM


# ---------------------------------------------------------------------------
# Host-side reference fallback (exact numpy port of the jax reference)
# ---------------------------------------------------------------------------
def _host_reference(fc_feats_target, fc_feats_distr, att_feats_target,
                    att_feats_distr, att_masks_target, att_masks_distr,
                    W1, b1, W2, b2, Wbil, oW1, ob1, oW2, ob2):
    def mlp(x, w1, bb1, w2, bb2):
        h = np.maximum(x @ w1 + bb1, 0.0)
        return (h @ w2 + bb2)[..., 0]

    ft = np.broadcast_to(fc_feats_target, fc_feats_distr.shape)
    cat = np.concatenate([ft, fc_feats_distr], axis=-1)
    scores = mlp(cat, W1, b1, W2, b2)
    tproj = np.einsum("bxd,de->bxe", att_feats_target[:, 0], Wbil)
    inners = np.einsum("bxd,biyd->bixy", tproj, att_feats_distr)
    mo = (att_masks_target[:, 0][:, None, :, None]
          * att_masks_distr[:, :, None, :])
    inners = np.where(mo > 0, inners, NEG)

    def sm(x):
        x = x - x.max(-1, keepdims=True)
        e = np.exp(x)
        return e / e.sum(-1, keepdims=True)

    tw = sm(inners.max(3))
    dw = sm(inners.max(2))
    tf = np.einsum("bxd,bix->bid", att_feats_target[:, 0], tw)
    df = np.einsum("biyd,biy->bid", att_feats_distr, dw)
    cat2 = np.concatenate([tf, df], axis=-1)
    scores = scores + mlp(cat2, oW1, ob1, oW2, ob2)
    ls = scores - scores.max(-1, keepdims=True)
    return (ls - np.log(np.exp(ls).sum(-1, keepdims=True))).astype(np.float32)


# ---------------------------------------------------------------------------
# NTFF trace support for test harnesses (KERNEL_TRACE=1)
# ---------------------------------------------------------------------------
def _install_trace_hook():
    import antenv

    if "antenv.axon_hooks" not in sys.modules:
        mod = types.ModuleType("antenv.axon_hooks")
        mod._hook = None

        def set_axon_ntff_profile_hook(h):
            mod._hook = h

        def get_axon_ntff_profile_hook():
            return mod._hook

        mod.set_axon_ntff_profile_hook = set_axon_ntff_profile_hook
        mod.get_axon_ntff_profile_hook = get_axon_ntff_profile_hook
        sys.modules["antenv.axon_hooks"] = mod
        antenv.axon_hooks = mod
    if sys.modules["antenv.axon_hooks"]._hook is None:
        from trn_agent_boot.trn_boot import _ntff_profile_via_ctypes

        hook = _ntff_profile_via_ctypes("/opt/axon/libaxon_pjrt.so")
        sys.modules["antenv.axon_hooks"].set_axon_ntff_profile_hook(hook)


# ---------------------------------------------------------------------------
# Host marshalling + entry point
# ---------------------------------------------------------------------------
def _prepare_in_maps(fc_feats_target, fc_feats_distr, att_feats_target,
                     att_feats_distr, att_masks_target, att_masks_distr,
                     W1, b1, W2, b2, oW1, ob1, oW2, ob2):
    fp = np.float32
    cat = np.concatenate(
        [np.broadcast_to(fc_feats_target, fc_feats_distr.shape), fc_feats_distr],
        axis=-1,
    ).astype(fp, copy=False)                       # [B, N, 2D]

    w1_t = np.ascontiguousarray(
        W1.astype(fp, copy=False).reshape(CH2, 128, H).transpose(1, 0, 2)
    ).astype(BF)
    oW1f = oW1.astype(fp, copy=False)
    ow1a_t = np.ascontiguousarray(
        oW1f[:D].reshape(CH, 128, H).transpose(1, 0, 2)).astype(BF)
    ow1b_t = np.ascontiguousarray(
        oW1f[D:].reshape(CH, 128, H).transpose(1, 0, 2))

    T = att_feats_target[:, 0].astype(fp, copy=False)       # [B, X, D]
    tt_all = np.ascontiguousarray(
        T.transpose(0, 2, 1).reshape(B, CH, 128, X).transpose(0, 2, 1, 3))
    tt2_all = tt_all.astype(BF)

    Dd = att_feats_distr.astype(fp, copy=False)             # [B, N, X, D]
    dt_all = np.ascontiguousarray(
        Dd.reshape(B, NG, GW, D).transpose(0, 1, 3, 2)
        .reshape(B, NG, CH, 128, GW).transpose(0, 1, 3, 2, 4))

    mtb = np.where(att_masks_target[:, 0] > 0, 0.0, NEG).astype(fp)  # [B, X]
    mdb = np.where(att_masks_distr > 0, 0.0, NEG).astype(fp)         # [B, N, X]

    b1_r = np.ascontiguousarray(b1.astype(fp).reshape(1, H)).astype(BF)
    ob1_r = np.ascontiguousarray(ob1.astype(fp).reshape(1, H))
    w2_r = np.ascontiguousarray(W2.astype(fp).reshape(1, H))
    ow2_r = np.ascontiguousarray(oW2.astype(fp).reshape(1, H))
    bias2 = np.ascontiguousarray(
        (b2.astype(np.float64) + ob2.astype(np.float64)).astype(fp).reshape(1, 1))

    in_maps = []
    for cc in range(NCORES):
        sl = slice(cc * BPC, (cc + 1) * BPC)
        cat_c = cat[sl].reshape(ROWS, 2 * D)
        cat_t = np.ascontiguousarray(
            cat_c.T.reshape(CH2, 128, ROWS).transpose(1, 0, 2)).astype(BF)
        in_maps.append({
            "cat_t": cat_t,
            "w1_t": w1_t,
            "ow1a_t": ow1a_t,
            "ow1b_t": ow1b_t,
            "tt_t": np.ascontiguousarray(tt_all[sl]),
            "tt2_t": np.ascontiguousarray(tt2_all[sl]),
            "dt_t": np.ascontiguousarray(dt_all[sl]),
            "b1_r": b1_r,
            "ob1_r": ob1_r,
            "w2_r": w2_r,
            "ow2_r": ow2_r,
            "bias2_r": bias2,
            "mtb": np.ascontiguousarray(mtb[sl].reshape(BPC, X, 1)),
            "mdb": np.ascontiguousarray(mdb[sl].reshape(BPC, 1, N * X)),
        })
    return in_maps


def kernel(**inputs):
    inp = {k: np.asarray(v) for k, v in inputs.items()}

    ident_ok = np.array_equal(
        inp["Wbil"], np.eye(D, dtype=inp["Wbil"].dtype))
    masks_ok = bool(
        (inp["att_masks_target"][:, 0] != 0).any(axis=1).all()
        and (inp["att_masks_distr"] != 0).any(axis=2).all())
    if not (ident_ok and masks_ok):
        return _host_reference(**inp)

    in_maps = _prepare_in_maps(
        inp["fc_feats_target"], inp["fc_feats_distr"],
        inp["att_feats_target"], inp["att_feats_distr"],
        inp["att_masks_target"], inp["att_masks_distr"],
        inp["W1"], inp["b1"], inp["W2"], inp["b2"],
        inp["oW1"], inp["ob1"], inp["oW2"], inp["ob2"])

    nc = _get_program()
    trace = os.environ.get("KERNEL_TRACE", "") == "1"
    if trace:
        _install_trace_hook()
        res = run_bass_kernel_spmd(
            nc, in_maps, list(range(NCORES)), trace=True,
            trace_cores=list(range(NCORES)))
        print(f"HW exec time: {res.exec_time_ns} ns")
    else:
        res = run_bass_kernel_spmd(nc, in_maps, list(range(NCORES)))

    out = np.concatenate(
        [res.results[cc]["out"].reshape(BPC, N) for cc in range(NCORES)], axis=0)
    return out.astype(np.float32, copy=False)
